# revision 14
# baseline (speedup 1.0000x reference)
"""Cross-attention layer on 8 TRN2 NeuronCores.

Sharding: core i -> (batch b = i//2, head-group g = i%2); each core computes
its head-group's contribution to out[b] through Wo; the host sums the two
partial products per batch (row-split of Wo => partial-sum reduction).

Device kernel works in transposed layout ([channels, tokens]) so the softmax
reduction is along the matmul free axis:
  Q^T = Wq_g^T x^T, K^T = Wk_g^T ctx^T, V' = [1 | pad | ctx Wv_g]
  scores^T_h = K_h Q_h^T  (contraction over head_dim=64)
  E = exp(scores^T/32) * mask^T      (no max subtraction; |scores/32| ~ 1.5)
  U = V'^T E  (per s-tile accumulation; row 0 = softmax denominator)
  O^T = U[64:128] * recip(U[0])      (DVE reciprocal_approx + gpsimd bcast)
  out_partial = O^T^T Wo_g           (host adds core pairs)

The kernel is paced by the ACT exp chain (~73us) and the HBM input stream
(~9MB at the ~200GB/s effective per-core rate under 8-core contention).
The schedule interleaves at matmul-group granularity: each attention unit's
four score groups are issued with one small filler (projection half, V
s-tile, PV unit or out-projection chunk) after each group, so the fixed
per-engine instruction order never head-of-line-blocks the exp chain, and
every DMA is issued in exact consumption order.

Dtype split: x/ctx/Wq/Wk/Wv, Q^T/K^T/V', probs, O^T, Wo and the out store
run in bf16; the mask is fp8 (exact 0/1); PSUM accumulation is fp32; the
softmax reciprocal is fp32 (DVE approx, ~51 ULP). Host sums pairs in fp32.
"""

import os
import numpy as np
import ml_dtypes

import concourse.mybir as mybir
from concourse import bacc
import concourse.tile as tile
from concourse.bass_utils import run_bass_kernel_spmd

B, T, TC = 4, 1024, 1024
C, CTX_C, H = 1024, 1024, 16
HD = C // H            # 64
P = 128
NCORES = 8
HG = 2                 # head groups
HPG = H // HG          # 8 heads per core
CG = HPG * HD          # 512 channels per group
NT = 512               # matmul free-dim chunk
KO = C // P            # 8 contraction tiles for projections
MQ = CG // P           # 4 partition-tiles of Q^T/K^T
SO = TC // P           # 8 s-tiles
T2 = T // NT           # 2 t-chunks
KP = CG // P           # 4 contraction tiles for the out projection
F32 = mybir.dt.float32
BF16 = mybir.dt.bfloat16
FP8 = mybir.dt.float8e4
ALU = mybir.AluOpType
ACTF = mybir.ActivationFunctionType

_CACHED_NC = None


def _ensure_ntff_hook():
    """Register the axon NTFF profiling hook if the image's antenv lacks it."""
    try:
        from antenv.axon_hooks import get_axon_ntff_profile_hook  # noqa: F401
        return
    except ImportError:
        pass
    import sys
    import types
    try:
        from trn_agent_boot.trn_boot import _ntff_profile_via_ctypes
        hook = _ntff_profile_via_ctypes("/opt/axon/libaxon_pjrt.so")
    except Exception:
        hook = None
    mod = types.ModuleType("antenv.axon_hooks")
    mod.get_axon_ntff_profile_hook = lambda: hook
    mod.set_axon_ntff_profile_hook = lambda h: None
    sys.modules["antenv.axon_hooks"] = mod
    import antenv
    antenv.axon_hooks = mod


def _hp(h):
    """Partition slice of local head h inside a [128, MQ, ...] channel tile."""
    lo = (h % 2) * HD
    return slice(lo, lo + HD)


def _build_program():
    nc = bacc.Bacc("TRN2", target_bir_lowering=False, debug=False,
                   num_devices=NCORES)
    xT = nc.dram_tensor("xT", [C, T], BF16, kind="ExternalInput").ap()
    ctxT = nc.dram_tensor("ctxT", [CTX_C, TC], BF16, kind="ExternalInput").ap()
    maskT = nc.dram_tensor("maskT", [TC, T], FP8, kind="ExternalInput").ap()
    # wq/wk host-pre-chunked: [MQ][P][KO][P] so one m-chunk is a single
    # contiguous-per-partition DMA (2KB lines).
    wqm = nc.dram_tensor("wqm", [MQ, P, KO, P], BF16, kind="ExternalInput").ap()
    wkm = nc.dram_tensor("wkm", [MQ, P, KO, P], BF16, kind="ExternalInput").ap()
    wv = nc.dram_tensor("wv", [CTX_C, CG], BF16, kind="ExternalInput").ap()
    wo = nc.dram_tensor("wo", [CG, C], BF16, kind="ExternalInput").ap()
    out = nc.dram_tensor("out", [T, C], BF16, kind="ExternalOutput").ap()

    with tile.TileContext(nc) as tc:
        with (
            tc.tile_pool(name="persist", bufs=1) as persist,
            tc.tile_pool(name="etp", bufs=7) as etp,
            tc.tile_pool(name="work", bufs=3) as work,
            tc.tile_pool(name="psmm", bufs=2, space="PSUM") as psmm,
            tc.tile_pool(name="pssc", bufs=2, space="PSUM") as pssc,
            tc.tile_pool(name="psu", bufs=2, space="PSUM") as psu_pool,
        ):
            qt_sb = persist.tile([P, MQ, T], BF16)            # Q^T [(h,d), t]
            kt_sb = persist.tile([P, MQ, TC], BF16)           # K^T [(h,d), s]
            vp_sb = persist.tile([P, SO, HPG, P], BF16)       # [1|pad63|V64]
            mask_sb = persist.tile([P, SO, T], FP8)           # mask^T
            ot_sb = persist.tile([P, KP, T], BF16)            # O^T normalized
            wo_sb = persist.tile([P, KP, C], BF16)
            xT_sb = persist.tile([P, KO, T], BF16)
            ctxT_sb = persist.tile([P, KO, TC], BF16)
            wq_sb = persist.tile([P, KO, CG], BF16)
            wk_sb = persist.tile([P, KO, CG], BF16)
            wv_sb = persist.tile([P, KO, CG], BF16)

            nc.gpsimd.memset(vp_sb[:, :, :, 0:1], 1.0)

            xT_r = xT.rearrange("(ko p) t -> p ko t", p=P)
            ctxT_r = ctxT.rearrange("(ko p) t -> p ko t", p=P)
            wv_r = wv.rearrange("(ko p) m -> p ko m", p=P)
            wo_r = wo.rearrange("(ko p) n -> p ko n", p=P)
            mask_r = maskT.rearrange("(so p) t -> p so t", p=P)

            def _mcols(m):
                return slice(m * P, (m + 1) * P)

            # DMAs: contiguous chunks, in exact consumption order.
            nc.sync.dma_start(wk_sb[:, :, 0:P], wkm[0])
            for kc in range(KO):       # ctxT s-half 0 -> kt s-tiles 0..3
                nc.sync.dma_start(ctxT_sb[:, kc, 0:NT], ctxT_r[:, kc, 0:NT])
            nc.sync.dma_start(wq_sb[:, :, 0:P], wqm[0])
            for kc in range(KO):
                nc.sync.dma_start(xT_sb[:, kc, 0:NT], xT_r[:, kc, 0:NT])
            for kc in range(KO):
                nc.sync.dma_start(ctxT_sb[:, kc, NT:T], ctxT_r[:, kc, NT:T])
            nc.sync.dma_start(wk_sb[:, :, _mcols(1)], wkm[1])
            nc.sync.dma_start(wq_sb[:, :, _mcols(1)], wqm[1])
            for kc in range(KO):
                nc.sync.dma_start(wv_sb[:, kc], wv_r[:, kc])
            nc.sync.dma_start(wk_sb[:, :, _mcols(2)], wkm[2])
            nc.sync.dma_start(wq_sb[:, :, _mcols(2)], wqm[2])
            for j in range(SO // 2):   # mask, t-half 0 (fp8: 128KB each)
                nc.sync.dma_start(mask_sb[:, 2 * j:2 * j + 2, 0:NT],
                                  mask_r[:, 2 * j:2 * j + 2, 0:NT])
            nc.sync.dma_start(wk_sb[:, :, _mcols(3)], wkm[3])
            nc.sync.dma_start(wq_sb[:, :, _mcols(3)], wqm[3])
            for kc in range(KO):
                nc.sync.dma_start(xT_sb[:, kc, NT:T], xT_r[:, kc, NT:T])
            for j in range(SO // 2):   # mask, t-half 1
                nc.sync.dma_start(mask_sb[:, 2 * j:2 * j + 2, NT:T],
                                  mask_r[:, 2 * j:2 * j + 2, NT:T])
            for kc in range(KO):
                nc.sync.dma_start(wo_sb[:, kc // 2, (kc % 2) * NT:
                                        (kc % 2) * NT + NT],
                                  wo_r[:, kc // 2, (kc % 2) * NT:
                                       (kc % 2) * NT + NT])

            # ---- small schedulable work chunks (~1-2us of PE each) ----
            def proj_B_half(m, s2):  # K^T chunk m, s half s2
                ps = psmm.tile([P, NT], F32, tag="mm512")
                for kc in range(KO):
                    nc.tensor.matmul(
                        ps, wk_sb[:, kc, _mcols(m)],
                        ctxT_sb[:, kc, s2 * NT:(s2 + 1) * NT],
                        start=(kc == 0), stop=(kc == KO - 1))
                nc.vector.tensor_copy(
                    kt_sb[:, m, s2 * NT:(s2 + 1) * NT], ps)

            def proj_A(m, t2):       # Q^T chunk m, t half t2
                ps = psmm.tile([P, NT], F32, tag="mm512")
                for kc in range(KO):
                    nc.tensor.matmul(
                        ps, wq_sb[:, kc, _mcols(m)],
                        xT_sb[:, kc, t2 * NT:(t2 + 1) * NT],
                        start=(kc == 0), stop=(kc == KO - 1))
                nc.vector.tensor_copy(
                    qt_sb[:, m, t2 * NT:(t2 + 1) * NT], ps)

            def proj_V(so):          # one V s-tile
                ps = psmm.tile([P, NT], F32, tag="mm512")
                for kc in range(KO):
                    nc.tensor.matmul(
                        ps, ctxT_sb[:, kc, so * P:(so + 1) * P],
                        wv_sb[:, kc, :],
                        start=(kc == 0), stop=(kc == KO - 1))
                nc.vector.tensor_copy(
                    vp_sb[:, so, :, 64:64 + HD],
                    ps.rearrange("p (h d) -> p h d", h=HPG))

            ets = {}

            def scores_unit(h, t2, fillers):
                et = etp.tile([P, SO, NT], BF16, tag="exp")
                ets[(h, t2)] = et
                tsl = slice(t2 * NT, (t2 + 1) * NT)
                for j in range(SO // 2):   # s-tile pairs share a 2-bank psum
                    ps = pssc.tile([P, 2 * NT], F32, tag="ps_sc")
                    for i in range(2):
                        so = 2 * j + i
                        nc.tensor.matmul(
                            ps[:, i * NT:(i + 1) * NT],
                            kt_sb[_hp(h), h // 2, so * P:(so + 1) * P],
                            qt_sb[_hp(h), h // 2, tsl],
                            start=True, stop=True)
                    nc.scalar.activation(
                        et[:, 2 * j:2 * j + 2, :].rearrange("p a b -> p (a b)"),
                        ps, ACTF.Exp, scale=1.0 / 32.0)
                    nc.vector.tensor_tensor(
                        et[:, 2 * j:2 * j + 2, :],
                        et[:, 2 * j:2 * j + 2, :],
                        mask_sb[:, 2 * j:2 * j + 2, tsl],
                        ALU.mult)
                    for f in fillers.get(j, []):
                        f()

            def pv_unit(h, t2):
                et = ets.pop((h, t2))
                psu = psu_pool.tile([P, NT], F32, tag="ps_u")
                for so in range(SO):
                    nc.tensor.matmul(
                        psu, vp_sb[:, so, h, :], et[:, so, :],
                        start=(so == 0), stop=(so == SO - 1))
                rc = work.tile([1, NT], F32, tag="recip")
                nc.vector.reciprocal_approx_fast(out=rc, in_=psu[0:1, :])
                bc = work.tile([HD, NT], F32, tag="bcast")
                nc.gpsimd.partition_broadcast(bc, rc)
                nc.vector.tensor_tensor(
                    ot_sb[_hp(h), h // 2, t2 * NT:(t2 + 1) * NT],
                    psu[64:64 + HD, :], bc, ALU.mult)

            def stage_D(tm, copies_on_act=False):
                for c2 in range(C // NT):
                    ps = psmm.tile([P, NT], F32, tag="mm512")
                    for kp in range(KP):
                        nc.tensor.matmul(
                            ps, ot_sb[:, kp, tm * P:(tm + 1) * P],
                            wo_sb[:, kp, c2 * NT:(c2 + 1) * NT],
                            start=(kp == 0), stop=(kp == KP - 1))
                    o_sb = work.tile([P, NT], BF16, tag="out")
                    if copies_on_act:
                        nc.scalar.activation(o_sb, ps, ACTF.Copy)
                    else:
                        nc.vector.tensor_copy(o_sb, ps)
                    nc.sync.dma_start(
                        out[tm * P:(tm + 1) * P, c2 * NT:(c2 + 1) * NT],
                        o_sb)

            # ---- schedule: 16 units t-major; one filler per score group ----
            def F(fn, *a, **kw):
                return lambda: fn(*a, **kw)

            fillers_by_unit = {
                0: {1: [F(proj_B_half, 0, 1)]},
                1: {0: [F(proj_B_half, 1, 0)], 1: [F(proj_B_half, 1, 1)],
                    2: [F(proj_A, 1, 0)]},
                2: {0: [F(proj_V, 0)], 1: [F(proj_V, 1)],
                    2: [F(proj_V, 2)], 3: [F(proj_V, 3)]},
                3: {0: [F(proj_B_half, 2, 0)], 1: [F(proj_B_half, 2, 1)],
                    2: [F(proj_A, 2, 0)], 3: [F(proj_V, 4)]},
                4: {0: [F(proj_V, 5)], 1: [F(proj_V, 6)],
                    2: [F(proj_V, 7)], 3: [F(pv_unit, 0, 0)]},
                5: {0: [F(proj_B_half, 3, 0)], 1: [F(proj_B_half, 3, 1)],
                    2: [F(proj_A, 3, 0)], 3: [F(pv_unit, 1, 0)]},
                6: {0: [F(pv_unit, 2, 0)], 1: [F(pv_unit, 3, 0)],
                    2: [F(proj_A, 0, 1)], 3: [F(pv_unit, 4, 0)]},
                7: {0: [F(proj_A, 1, 1)], 1: [F(pv_unit, 5, 0)],
                    2: [F(proj_A, 2, 1)], 3: [F(proj_A, 3, 1)]},
                8: {0: [F(pv_unit, 6, 0)], 1: [F(pv_unit, 7, 0)],
                    2: [F(stage_D, 0)], 3: [F(stage_D, 1)]},
                9: {0: [F(stage_D, 2)], 1: [F(stage_D, 3)],
                    2: [F(pv_unit, 0, 1)]},
                10: {1: [F(pv_unit, 1, 1)]},
                11: {1: [F(pv_unit, 2, 1)]},
                12: {1: [F(pv_unit, 3, 1)]},
                13: {1: [F(pv_unit, 4, 1)]},
                14: {1: [F(pv_unit, 5, 1)]},
                15: {1: [F(pv_unit, 6, 1)]},
            }

            proj_B_half(0, 0)
            proj_A(0, 0)
            units = [(h, t2) for t2 in range(T2) for h in range(HPG)]
            for u, (h, t2) in enumerate(units):
                scores_unit(h, t2, fillers_by_unit.get(u, {}))
            pv_unit(7, 1)
            for tm in range(4, 8):
                stage_D(tm, copies_on_act=(tm >= 6))
    nc.compile()
    return nc


def _get_program():
    global _CACHED_NC
    if _CACHED_NC is None:
        _CACHED_NC = _build_program()
    return _CACHED_NC


def kernel(x, context, attn_mask, Wq, Wk, Wv, Wo):
    x = np.asarray(x, dtype=np.float32)
    context = np.asarray(context, dtype=np.float32)
    attn_mask = np.asarray(attn_mask)
    Wq = np.asarray(Wq, dtype=np.float32)
    Wk = np.asarray(Wk, dtype=np.float32)
    Wv = np.asarray(Wv, dtype=np.float32)
    Wo = np.asarray(Wo, dtype=np.float32)

    nc = _get_program()
    bf = ml_dtypes.bfloat16

    def _mchunk(w):
        # [C, CG_slice] -> [MQ, P, KO, P]: per column-block, partition-major
        return np.ascontiguousarray(
            w.reshape(KO, P, MQ, P).transpose(2, 1, 0, 3)).astype(bf)

    in_maps = []
    for i in range(NCORES):
        b, g = i // 2, i % 2
        cs = slice(g * CG, (g + 1) * CG)
        in_maps.append({
            "xT": np.ascontiguousarray(x[b].T).astype(bf),
            "ctxT": np.ascontiguousarray(context[b].T).astype(bf),
            "maskT": np.ascontiguousarray(attn_mask[b, 0].T).astype(
                ml_dtypes.float8_e4m3),
            "wqm": _mchunk(Wq[:, cs]),
            "wkm": _mchunk(Wk[:, cs]),
            "wv": np.ascontiguousarray(Wv[:, cs]).astype(bf),
            "wo": np.ascontiguousarray(Wo[cs, :]).astype(bf),
        })

    profile = os.environ.get("KERNEL_PROFILE", "0") == "1"
    if profile:
        _ensure_ntff_hook()
    res = run_bass_kernel_spmd(
        nc, in_maps, list(range(NCORES)),
        trace=profile, trace_cores=[0] if profile else None)
    if profile:
        kernel.last_exec_time_ns = res.exec_time_ns
        kernel.last_trace = res.instructions_and_trace

    out = np.empty((B, T, C), dtype=np.float32)
    for b in range(B):
        out[b] = (res.results[2 * b]["out"].astype(np.float32)
                  + res.results[2 * b + 1]["out"].astype(np.float32))
    return out


# revision 16
# speedup vs baseline: 1.1290x; 1.1290x over previous
"""Cross-attention layer on 8 TRN2 NeuronCores.

Sharding: core i -> (batch b = i//2, head-group g = i%2); each core computes
its head-group's contribution to out[b] through Wo; the host sums the two
partial products per batch (row-split of Wo => partial-sum reduction).

Device kernel works in transposed layout ([channels, tokens]) so the softmax
reduction is along the matmul free axis:
  Q^T = Wq_g^T x^T, K^T = Wk_g^T ctx^T, V' = [1 | pad | ctx Wv_g]
  scores^T_h = K_h Q_h^T  (contraction over head_dim=64; head pairs issued
                           interleaved so the PE row-halves overlap)
  E = exp(scores^T/32) * mask^T      (no max subtraction; |scores/32| ~ 1.5)
  U = V'^T E  (per s-tile accumulation; row 0 = softmax denominator)
  O^T = U[64:128] * recip(U[0])      (DVE reciprocal_approx + gpsimd bcast)
  out_partial = O^T^T Wo_g           (host adds core pairs)

The kernel is PE-bound (~110us of matmul work vs a ~73us ACT exp chain), so
the fixed per-engine issue order is arranged to keep the PE dense: warm-up
matmuls on scratch data cover the DMA head (and hold off HAM re-throttle),
every DMA is issued in exact consumption order, and projections / V /
PV / out-projection chunks are placed so their inputs are ready before the
PE reaches them.

Dtype split: x/ctx/Wq/Wk/Wv/mask, Q^T/K^T/V', probs, O^T, Wo and the out
store run in bf16; PSUM accumulation is fp32; the softmax reciprocal is
fp32 (DVE approx, ~51 ULP).  Host sums the core pairs in fp32.
"""

import os
import numpy as np
import ml_dtypes

import concourse.mybir as mybir
from concourse import bacc
import concourse.tile as tile
from concourse.bass_utils import run_bass_kernel_spmd

B, T, TC = 4, 1024, 1024
C, CTX_C, H = 1024, 1024, 16
HD = C // H            # 64
P = 128
NCORES = 8
HG = 2                 # head groups
HPG = H // HG          # 8 heads per core
CG = HPG * HD          # 512 channels per group
NT = 512               # matmul free-dim chunk
KO = C // P            # 8 contraction tiles for projections
MQ = CG // P           # 4 partition-tiles of Q^T/K^T
SO = TC // P           # 8 s-tiles
T2 = T // NT           # 2 t-chunks
KP = CG // P           # 4 contraction tiles for the out projection
F32 = mybir.dt.float32
BF16 = mybir.dt.bfloat16
ALU = mybir.AluOpType
ACTF = mybir.ActivationFunctionType

_CACHED_NC = None


def _ensure_ntff_hook():
    """Register the axon NTFF profiling hook if the image's antenv lacks it."""
    try:
        from antenv.axon_hooks import get_axon_ntff_profile_hook  # noqa: F401
        return
    except ImportError:
        pass
    import sys
    import types
    try:
        from trn_agent_boot.trn_boot import _ntff_profile_via_ctypes
        hook = _ntff_profile_via_ctypes("/opt/axon/libaxon_pjrt.so")
    except Exception:
        hook = None
    mod = types.ModuleType("antenv.axon_hooks")
    mod.get_axon_ntff_profile_hook = lambda: hook
    mod.set_axon_ntff_profile_hook = lambda h: None
    sys.modules["antenv.axon_hooks"] = mod
    import antenv
    antenv.axon_hooks = mod


def _hp(h):
    """Partition slice of local head h inside a [128, MQ, ...] channel tile."""
    lo = (h % 2) * HD
    return slice(lo, lo + HD)


def _build_program():
    nc = bacc.Bacc("TRN2", target_bir_lowering=False, debug=False,
                   num_devices=NCORES)
    xT = nc.dram_tensor("xT", [C, T], BF16, kind="ExternalInput").ap()
    ctxT = nc.dram_tensor("ctxT", [CTX_C, TC], BF16, kind="ExternalInput").ap()
    maskT = nc.dram_tensor("maskT", [TC, T], BF16, kind="ExternalInput").ap()
    # wq/wk host-pre-chunked: [MQ][P][KO][P] so one m-chunk is a single
    # contiguous-per-partition DMA (2KB lines).
    wqm = nc.dram_tensor("wqm", [MQ, P, KO, P], BF16, kind="ExternalInput").ap()
    wkm = nc.dram_tensor("wkm", [MQ, P, KO, P], BF16, kind="ExternalInput").ap()
    wv = nc.dram_tensor("wv", [CTX_C, CG], BF16, kind="ExternalInput").ap()
    wo = nc.dram_tensor("wo", [CG, C], BF16, kind="ExternalInput").ap()
    out = nc.dram_tensor("out", [T, C], BF16, kind="ExternalOutput").ap()

    with tile.TileContext(nc) as tc:
        with (
            tc.tile_pool(name="persist", bufs=1) as persist,
            tc.tile_pool(name="etp", bufs=6) as etp,
            tc.tile_pool(name="work", bufs=3) as work,
            tc.tile_pool(name="psmm", bufs=2, space="PSUM") as psmm,
            tc.tile_pool(name="pssc", bufs=2, space="PSUM") as pssc,
            tc.tile_pool(name="psu", bufs=2, space="PSUM") as psu_pool,
        ):
            qt_sb = persist.tile([P, MQ, T], BF16)            # Q^T [(h,d), t]
            kt_sb = persist.tile([P, MQ, TC], BF16)           # K^T [(h,d), s]
            vp_sb = persist.tile([P, SO, HPG, P], BF16)       # [1|pad63|V64]
            mask_sb = persist.tile([P, SO, T], BF16)          # mask^T
            ot_sb = persist.tile([P, KP, T], BF16)            # O^T normalized
            wo_sb = persist.tile([P, KP, C], BF16)
            xT_sb = persist.tile([P, KO, T], BF16)
            ctxT_sb = persist.tile([P, KO, TC], BF16)
            wq_sb = persist.tile([P, KO, CG], BF16)
            wk_sb = persist.tile([P, KO, CG], BF16)
            wv_sb = persist.tile([P, KO, CG], BF16)
            warm_sb = persist.tile([P, NT], BF16)

            nc.vector.memset(warm_sb, 0.25)
            nc.gpsimd.memset(vp_sb[:, :, :, 0:1], 1.0)

            # ACT table preload + PE warm-up during the DMA head: ~36 dummy
            # matmuls keep the PE HAM un-throttled until real data lands.
            wps = psmm.tile([P, NT], F32, tag="mm512")
            wet = work.tile([1, NT], BF16, tag="warmout")
            for w in range(36):
                nc.tensor.matmul(wps, warm_sb[:, 0:P], warm_sb,
                                 start=True, stop=True)
            nc.scalar.activation(wet, wps[0:1, :], ACTF.Exp, scale=1.0 / 32.0)

            xT_r = xT.rearrange("(ko p) t -> p ko t", p=P)
            ctxT_r = ctxT.rearrange("(ko p) t -> p ko t", p=P)
            wv_r = wv.rearrange("(ko p) m -> p ko m", p=P)
            wo_r = wo.rearrange("(ko p) n -> p ko n", p=P)
            mask_r = maskT.rearrange("(so p) t -> p so t", p=P)

            def _mcols(m):
                return slice(m * P, (m + 1) * P)

            # DMAs: contiguous chunks, in exact consumption order.
            nc.sync.dma_start(wk_sb[:, :, 0:P], wkm[0])
            for kc in range(KO):       # ctxT s-half 0 -> kt s-tiles 0..3
                nc.sync.dma_start(ctxT_sb[:, kc, 0:NT], ctxT_r[:, kc, 0:NT])
            nc.sync.dma_start(wq_sb[:, :, 0:P], wqm[0])
            for kc in range(KO):
                nc.sync.dma_start(xT_sb[:, kc, 0:NT], xT_r[:, kc, 0:NT])
            for kc in range(KO):
                nc.sync.dma_start(ctxT_sb[:, kc, NT:T], ctxT_r[:, kc, NT:T])
            nc.sync.dma_start(wk_sb[:, :, _mcols(1)], wkm[1])
            nc.sync.dma_start(wq_sb[:, :, _mcols(1)], wqm[1])
            for kc in range(KO):
                nc.sync.dma_start(wv_sb[:, kc], wv_r[:, kc])
            for j in range(SO // 2):   # mask, t-half 0
                nc.sync.dma_start(mask_sb[:, 2 * j:2 * j + 2, 0:NT],
                                  mask_r[:, 2 * j:2 * j + 2, 0:NT])
            nc.sync.dma_start(wk_sb[:, :, _mcols(2)], wkm[2])
            nc.sync.dma_start(wq_sb[:, :, _mcols(2)], wqm[2])
            nc.sync.dma_start(wk_sb[:, :, _mcols(3)], wkm[3])
            nc.sync.dma_start(wq_sb[:, :, _mcols(3)], wqm[3])
            for kc in range(KO):
                nc.sync.dma_start(xT_sb[:, kc, NT:T], xT_r[:, kc, NT:T])
            for j in range(SO // 2):   # mask, t-half 1
                nc.sync.dma_start(mask_sb[:, 2 * j:2 * j + 2, NT:T],
                                  mask_r[:, 2 * j:2 * j + 2, NT:T])
            for kc in range(KO):
                nc.sync.dma_start(wo_sb[:, kc // 2, (kc % 2) * NT:
                                        (kc % 2) * NT + NT],
                                  wo_r[:, kc // 2, (kc % 2) * NT:
                                       (kc % 2) * NT + NT])

            # ---- schedulable work chunks ----
            def proj_B_half(m, s2):
                ps = psmm.tile([P, NT], F32, tag="mm512")
                for kc in range(KO):
                    nc.tensor.matmul(
                        ps, wk_sb[:, kc, _mcols(m)],
                        ctxT_sb[:, kc, s2 * NT:(s2 + 1) * NT],
                        start=(kc == 0), stop=(kc == KO - 1))
                nc.vector.tensor_copy(
                    kt_sb[:, m, s2 * NT:(s2 + 1) * NT], ps)

            def proj_A(m, t2):
                ps = psmm.tile([P, NT], F32, tag="mm512")
                for kc in range(KO):
                    nc.tensor.matmul(
                        ps, wq_sb[:, kc, _mcols(m)],
                        xT_sb[:, kc, t2 * NT:(t2 + 1) * NT],
                        start=(kc == 0), stop=(kc == KO - 1))
                nc.vector.tensor_copy(
                    qt_sb[:, m, t2 * NT:(t2 + 1) * NT], ps)

            def proj_V(so_lo, so_hi):
                for so in range(so_lo, so_hi):
                    ps = psmm.tile([P, NT], F32, tag="mm512")
                    for kc in range(KO):
                        nc.tensor.matmul(
                            ps, ctxT_sb[:, kc, so * P:(so + 1) * P],
                            wv_sb[:, kc, :],
                            start=(kc == 0), stop=(kc == KO - 1))
                    nc.vector.tensor_copy(
                        vp_sb[:, so, :, 64:64 + HD],
                        ps.rearrange("p (h d) -> p h d", h=HPG))

            ets = {}

            def scores_pair(hp, t2):
                h0, h1 = 2 * hp, 2 * hp + 1
                etA = etp.tile([P, SO, NT], BF16, tag="exp")
                etB = etp.tile([P, SO, NT], BF16, tag="exp")
                ets[(h0, t2)] = etA
                ets[(h1, t2)] = etB
                tsl = slice(t2 * NT, (t2 + 1) * NT)
                for j in range(SO // 2):
                    psA = pssc.tile([P, 2 * NT], F32, tag="ps_sc")
                    psB = pssc.tile([P, 2 * NT], F32, tag="ps_sc")
                    for i in range(2):
                        so = 2 * j + i
                        ssl = slice(so * P, (so + 1) * P)
                        nc.tensor.matmul(
                            psA[:, i * NT:(i + 1) * NT],
                            kt_sb[_hp(h0), hp, ssl], qt_sb[_hp(h0), hp, tsl],
                            start=True, stop=True)
                        nc.tensor.matmul(
                            psB[:, i * NT:(i + 1) * NT],
                            kt_sb[_hp(h1), hp, ssl], qt_sb[_hp(h1), hp, tsl],
                            start=True, stop=True)
                    msl = mask_sb[:, 2 * j:2 * j + 2, tsl]
                    for et, ps in ((etA, psA), (etB, psB)):
                        nc.scalar.activation(
                            et[:, 2 * j:2 * j + 2, :].rearrange(
                                "p a b -> p (a b)"),
                            ps, ACTF.Exp, scale=1.0 / 32.0)
                        nc.vector.tensor_tensor(
                            et[:, 2 * j:2 * j + 2, :],
                            et[:, 2 * j:2 * j + 2, :], msl, ALU.mult)

            def pv_unit(h, t2):
                et = ets.pop((h, t2))
                psu = psu_pool.tile([P, NT], F32, tag="ps_u")
                for so in range(SO):
                    nc.tensor.matmul(
                        psu, vp_sb[:, so, h, :], et[:, so, :],
                        start=(so == 0), stop=(so == SO - 1))
                rc = work.tile([1, NT], F32, tag="recip")
                nc.vector.reciprocal_approx_fast(out=rc, in_=psu[0:1, :])
                bc = work.tile([HD, NT], F32, tag="bcast")
                nc.gpsimd.partition_broadcast(bc, rc)
                nc.vector.tensor_tensor(
                    ot_sb[_hp(h), h // 2, t2 * NT:(t2 + 1) * NT],
                    psu[64:64 + HD, :], bc, ALU.mult)

            def stage_D(tm_lo, tm_hi, copies_on_act=False):
                for tm in range(tm_lo, tm_hi):
                    for c2 in range(C // NT):
                        ps = psmm.tile([P, NT], F32, tag="mm512")
                        for kp in range(KP):
                            nc.tensor.matmul(
                                ps, ot_sb[:, kp, tm * P:(tm + 1) * P],
                                wo_sb[:, kp, c2 * NT:(c2 + 1) * NT],
                                start=(kp == 0), stop=(kp == KP - 1))
                        o_sb = work.tile([P, NT], BF16, tag="out")
                        if copies_on_act:
                            nc.scalar.activation(o_sb, ps, ACTF.Copy)
                        else:
                            nc.vector.tensor_copy(o_sb, ps)
                        nc.sync.dma_start(
                            out[tm * P:(tm + 1) * P, c2 * NT:(c2 + 1) * NT],
                            o_sb)

            # ---- schedule (fixed per-engine order = issue order) ----
            proj_B_half(0, 0)
            proj_A(0, 0)
            proj_B_half(0, 1)
            scores_pair(0, 0)            # heads 0,1 @ t0
            proj_B_half(1, 0)
            proj_B_half(1, 1)
            proj_A(1, 0)
            scores_pair(1, 0)            # heads 2,3
            proj_V(0, 4)
            proj_V(4, 8)
            pv_unit(0, 0)
            pv_unit(1, 0)
            proj_B_half(2, 0)
            proj_B_half(2, 1)
            proj_A(2, 0)
            scores_pair(2, 0)            # heads 4,5
            pv_unit(2, 0)
            pv_unit(3, 0)
            proj_B_half(3, 0)
            proj_B_half(3, 1)
            proj_A(3, 0)
            scores_pair(3, 0)            # heads 6,7
            pv_unit(4, 0)
            pv_unit(5, 0)
            proj_A(0, 1)
            proj_A(1, 1)
            scores_pair(0, 1)            # heads 0,1 @ t1
            pv_unit(6, 0)
            pv_unit(7, 0)
            stage_D(0, 2)
            proj_A(2, 1)
            proj_A(3, 1)
            scores_pair(1, 1)            # heads 2,3
            pv_unit(0, 1)
            pv_unit(1, 1)
            stage_D(2, 4)
            scores_pair(2, 1)            # heads 4,5
            pv_unit(2, 1)
            pv_unit(3, 1)
            scores_pair(3, 1)            # heads 6,7
            pv_unit(4, 1)
            pv_unit(5, 1)
            pv_unit(6, 1)
            pv_unit(7, 1)
            stage_D(4, 8, copies_on_act=True)
    nc.compile()
    return nc


def _get_program():
    global _CACHED_NC
    if _CACHED_NC is None:
        _CACHED_NC = _build_program()
    return _CACHED_NC


def kernel(x, context, attn_mask, Wq, Wk, Wv, Wo):
    x = np.asarray(x, dtype=np.float32)
    context = np.asarray(context, dtype=np.float32)
    attn_mask = np.asarray(attn_mask)
    Wq = np.asarray(Wq, dtype=np.float32)
    Wk = np.asarray(Wk, dtype=np.float32)
    Wv = np.asarray(Wv, dtype=np.float32)
    Wo = np.asarray(Wo, dtype=np.float32)

    nc = _get_program()
    bf = ml_dtypes.bfloat16

    def _mchunk(w):
        # [C, CG_slice] -> [MQ, P, KO, P]: per column-block, partition-major
        return np.ascontiguousarray(
            w.reshape(KO, P, MQ, P).transpose(2, 1, 0, 3)).astype(bf)

    in_maps = []
    for i in range(NCORES):
        b, g = i // 2, i % 2
        cs = slice(g * CG, (g + 1) * CG)
        in_maps.append({
            "xT": np.ascontiguousarray(x[b].T).astype(bf),
            "ctxT": np.ascontiguousarray(context[b].T).astype(bf),
            "maskT": np.ascontiguousarray(attn_mask[b, 0].T).astype(bf),
            "wqm": _mchunk(Wq[:, cs]),
            "wkm": _mchunk(Wk[:, cs]),
            "wv": np.ascontiguousarray(Wv[:, cs]).astype(bf),
            "wo": np.ascontiguousarray(Wo[cs, :]).astype(bf),
        })

    profile = os.environ.get("KERNEL_PROFILE", "0") == "1"
    if profile:
        _ensure_ntff_hook()
    res = run_bass_kernel_spmd(
        nc, in_maps, list(range(NCORES)),
        trace=profile, trace_cores=[0] if profile else None)
    if profile:
        kernel.last_exec_time_ns = res.exec_time_ns
        kernel.last_trace = res.instructions_and_trace

    out = np.empty((B, T, C), dtype=np.float32)
    for b in range(B):
        out[b] = (res.results[2 * b]["out"].astype(np.float32)
                  + res.results[2 * b + 1]["out"].astype(np.float32))
    return out
